# revision 1
# baseline (speedup 1.0000x reference)
"""NoisyTopKRouter (eval mode) on 8 Trainium2 NeuronCores.

Computes, for x [4, 8192, 2048] fp32 and W_gate [2048, 8] fp32, top_k=2:
    h = x_flat @ W_gate                       (fp32-accurate, see below)
    dispatch_weights = softmax(top-2 logits)  [4, 8192, 2]
    top_k_idx        = top-2 indices          [4, 8192, 2] int32
    auxiliary_loss   = cv(load) + cv(importance) scaled

Sharding: data-parallel over the flattened token dim, 4096 tokens per core;
W replicated. Per-core aux-stat partials (importance sums) are reduced on the
host; `load` comes from the returned indices.

Device kernel (per 128-token tile):
  - PE transposes the fp32 x tile (d must sit on partitions for the matmul).
  - The transposed fp32 tile in PSUM is split into an exact fp16 hi/lo pair
    (ACT cast + DVE subtract), and h is computed as the 4-term product
    (hi+lo) @ (W_hi+W_lo) with bf16-speed PE matmuls and fp32 PSUM
    accumulation - accuracy is fp32-class (~1e-6) at ~4x fp32 PE speed.
  - DVE max8/max_index give a full descending sort of the 8 logits; ACT Exp
    (+accumulator) and a few small DVE ops produce softmax weights and
    per-token probabilities accumulated into the importance partials.
The host additionally re-verifies tokens whose top-2/top-3 margins are within
a small threshold by recomputing those rows exactly.
"""
import os
import sys

for _p in ("/opt/trn_rl_repo", "/root/.axon_site/_ro/trn_rl_repo"):
    if os.path.isdir(_p) and _p not in sys.path:
        sys.path.append(_p)

import numpy as np

import concourse.bass as bass  # noqa: E402
import concourse.mybir as mybir  # noqa: E402
import concourse.tile as tile  # noqa: E402
from concourse import bacc  # noqa: E402
from concourse.bass_utils import run_bass_kernel_spmd  # noqa: E402
from concourse.masks import make_identity  # noqa: E402

F32 = mybir.dt.float32
F16 = mybir.dt.float16
U32 = mybir.dt.uint32
Exp = mybir.ActivationFunctionType.Exp
Copy = mybir.ActivationFunctionType.Copy

NCORES = 8
D = 2048
E = 8
KT = D // 128
T = 128
GROUP = 4
NTOK_TOTAL = 4 * 8192
NTOK = NTOK_TOTAL // NCORES

NOISE_STD = 0.01
AUX_COEF = 0.01
EPS = 1e-6

MARGIN_TAU = 3e-4  # host re-check threshold on logit margins


def _build_router(n_tok: int):
    assert n_tok % (T * GROUP) == 0
    n_tiles = n_tok // T
    n_groups = n_tiles // GROUP

    nc = bacc.Bacc(None, target_bir_lowering=False)
    x_d = nc.dram_tensor("x", [n_tok, D], F32, kind="ExternalInput")
    wp_d = nc.dram_tensor("wpair", [D, 2 * E], F16, kind="ExternalInput")
    wout_d = nc.dram_tensor("w_out", [n_tok, 2], F32, kind="ExternalOutput")
    iout_d = nc.dram_tensor("i_out", [n_tok, 2], U32, kind="ExternalOutput")
    imp_d = nc.dram_tensor("imp_out", [128, E], F32, kind="ExternalOutput")
    m23_d = nc.dram_tensor("m23_out", [128, n_tiles, 2], F32, kind="ExternalOutput")

    with tile.TileContext(nc) as tc:
        with (
            tc.tile_pool(name="const", bufs=1) as cpool,
            tc.tile_pool(name="xin", bufs=4) as xpool,
            tc.tile_pool(name="xt", bufs=3) as xtpool,
            tc.tile_pool(name="small", bufs=3) as spool,
            tc.tile_pool(name="stage", bufs=3) as stpool,
            tc.tile_pool(name="pst", bufs=4, space="PSUM") as pst_pool,
            tc.tile_pool(name="psh", bufs=3, space="PSUM") as psh_pool,
        ):
            ident = cpool.tile([128, 128], F32)
            make_identity(nc, ident)
            wp_sb = cpool.tile([128, KT, 2 * E], F16)
            nc.sync.dma_start(wp_sb[:], wp_d.rearrange("(k p) e -> p k e", p=128))

            acc_imp = cpool.tile([128, E], F32)
            nc.vector.memset(acc_imp[:], 0.0)

            for g in range(n_groups):
                wstg = stpool.tile([128, GROUP, 2], F32, tag="wstg")
                istg = stpool.tile([128, GROUP, 2], U32, tag="istg")
                mstg = stpool.tile([128, GROUP, 2], F32, tag="mstg")
                h4 = spool.tile([128, GROUP, E], F32)
                mx4 = spool.tile([128, GROUP, E], F32)
                mi4 = spool.tile([128, GROUP, E], U32)
                p4 = spool.tile([128, GROUP, E], F32)
                z4 = spool.tile([128, GROUP], F32)
                sc4 = spool.tile([128, 4 * GROUP], F32)
                neg4 = sc4[:, 0:GROUP]
                rz4 = sc4[:, GROUP:2 * GROUP]
                d21 = sc4[:, 2 * GROUP:3 * GROUP]
                e2 = sc4[:, 3 * GROUP:4 * GROUP]
                t8 = spool.tile([128, GROUP, E], F32, tag="t8")

                for ti in range(GROUP):
                    t = g * GROUP + ti
                    x_sb = xpool.tile([128, D], F32)
                    nc.sync.dma_start(x_sb[:], x_d[t * T:(t + 1) * T, :])

                    hi = xtpool.tile([128, D], F16, tag="hi")
                    lo = xtpool.tile([128, D], F16, tag="lo")
                    for kk in range(4):
                        ps = pst_pool.tile([128, 512], F32)
                        for j in range(4):
                            k = kk * 4 + j
                            nc.tensor.transpose(
                                ps[:, j * 128:(j + 1) * 128],
                                x_sb[:, k * 128:(k + 1) * 128],
                                ident[:],
                            )
                        sl = slice(kk * 512, (kk + 1) * 512)
                        nc.scalar.activation(hi[:, sl], ps[:], Copy)
                        nc.vector.tensor_sub(lo[:, sl], ps[:], hi[:, sl])

                    h_ps = psh_pool.tile([128, 2 * E], F32)
                    for k in range(KT):
                        nc.tensor.matmul(
                            h_ps[:], hi[:, k * 128:(k + 1) * 128], wp_sb[:, k, :],
                            start=(k == 0), stop=False,
                        )
                    for k in range(KT):
                        nc.tensor.matmul(
                            h_ps[:], lo[:, k * 128:(k + 1) * 128], wp_sb[:, k, :],
                            start=False, stop=(k == KT - 1),
                        )
                    # fold the two 8-wide halves: h = x@W_hi + x@W_lo
                    nc.vector.tensor_copy(t8[:, ti, :], h_ps[:, E:2 * E])
                    nc.vector.tensor_add(h4[:, ti, :], h_ps[:, 0:E], t8[:, ti, :])

                    nc.vector.max(out=mx4[:, ti, :], in_=h4[:, ti, :])
                    nc.vector.max_index(
                        out=mi4[:, ti, :], in_max=mx4[:, ti, :], in_values=h4[:, ti, :]
                    )

                # batched per-group router math
                nc.vector.tensor_scalar_mul(neg4, mx4[:, :, 0], -1.0)
                for ti in range(GROUP):
                    nc.scalar.activation(
                        p4[:, ti, :], h4[:, ti, :], Exp,
                        bias=neg4[:, ti:ti + 1], accum_out=z4[:, ti:ti + 1],
                    )
                nc.vector.reciprocal(rz4, z4[:])
                for ti in range(GROUP):
                    nc.vector.tensor_scalar_mul(
                        p4[:, ti, :], p4[:, ti, :], rz4[:, ti:ti + 1])
                for ti in range(GROUP):
                    nc.vector.tensor_add(acc_imp[:], acc_imp[:], p4[:, ti, :])

                nc.vector.tensor_sub(d21, mx4[:, :, 1], mx4[:, :, 0])
                nc.scalar.activation(e2, d21, Exp)
                den = d21
                nc.vector.tensor_scalar_add(den, e2, 1.0)
                nc.vector.reciprocal(wstg[:, :, 0], den)
                nc.vector.tensor_mul(wstg[:, :, 1], e2, wstg[:, :, 0])
                nc.vector.tensor_copy(istg[:], mi4[:, :, 0:2])
                nc.vector.tensor_copy(mstg[:], mx4[:, :, 1:3])

                # stream group outputs on the SWDGE ring (overlaps compute)
                ggs = slice(g * GROUP * T, (g + 1) * GROUP * T)
                nc.gpsimd.dma_start(
                    wout_d[ggs, :].rearrange("(gg p) t -> p gg t", p=128), wstg[:])
                nc.gpsimd.dma_start(
                    iout_d[ggs, :].rearrange("(gg p) t -> p gg t", p=128), istg[:])
                nc.gpsimd.dma_start(m23_d[:, g * GROUP:(g + 1) * GROUP, :], mstg[:])

            nc.gpsimd.dma_start(imp_d[:], acc_imp[:])

    nc.compile()
    return nc


_NC_CACHE = {}


def _get_nc():
    if "nc" not in _NC_CACHE:
        _NC_CACHE["nc"] = _build_router(NTOK)
    return _NC_CACHE["nc"]


def _make_wpair(w_gate):
    w_hi = w_gate.astype(np.float16)
    w_lo = (w_gate.astype(np.float64) - w_hi.astype(np.float64)).astype(np.float16)
    return np.ascontiguousarray(np.concatenate([w_hi, w_lo], axis=1))


def kernel(x, W_gate, W_noise, top_k):
    tk = int(np.asarray(top_k))
    assert tk == 2, f"kernel hardcodes top_k=2, got {tk}"
    x = np.asarray(x, dtype=np.float32)
    W_gate = np.asarray(W_gate, dtype=np.float32)
    assert x.shape == (4, 8192, D) and W_gate.shape == (D, E)

    x_flat = np.ascontiguousarray(x.reshape(-1, D))
    wp = _make_wpair(W_gate)

    nc = _get_nc()
    in_maps = [
        {"x": x_flat[c * NTOK:(c + 1) * NTOK], "wpair": wp} for c in range(NCORES)
    ]
    res = run_bass_kernel_spmd(nc, in_maps, core_ids=list(range(NCORES)))

    wts = np.concatenate([res.results[c]["w_out"] for c in range(NCORES)])
    idx = np.concatenate([res.results[c]["i_out"] for c in range(NCORES)])
    idx = idx.astype(np.int32)
    importance = np.zeros(E, dtype=np.float64)
    for c in range(NCORES):
        importance += res.results[c]["imp_out"].astype(np.float64).sum(axis=0)
    # m23_out [128, n_tiles, 2] per core -> token order: tile-major, partition
    m23 = np.concatenate([
        res.results[c]["m23_out"].transpose(1, 0, 2).reshape(-1, 2)
        for c in range(NCORES)
    ])

    # Host re-verification of near-tie tokens: margins m1-m2 (from the
    # weights) and m2-m3 (from m23) below tau get recomputed exactly.
    gap23 = m23[:, 0] - m23[:, 1]
    r = np.clip(wts[:, 1], 1e-30, 1.0) / np.clip(wts[:, 0], 1e-30, 1.0)
    gap12 = -np.log(r)  # m2 - m1 is -gap12
    suspect = np.where((gap23 < MARGIN_TAU) | (gap12 < MARGIN_TAU))[0]
    if suspect.size:
        xs = x_flat[suspect].astype(np.float64)
        hs = xs @ W_gate.astype(np.float64)
        order = np.argsort(-hs, axis=1, kind="stable")[:, :2]
        top2 = np.take_along_axis(hs, order, axis=1)
        ee = np.exp(top2 - top2[:, :1])
        ww = (ee / ee.sum(axis=1, keepdims=True)).astype(np.float32)
        idx[suspect] = order.astype(np.int32)
        wts[suspect] = ww

    load = np.bincount(idx.reshape(-1), minlength=E).astype(np.float64)

    def _cv(v):
        return np.std(v, ddof=1) / (v.mean() + EPS)

    aux = np.float32((_cv(load) + _cv(importance)) * AUX_COEF)

    dispatch_weights = wts.reshape(4, 8192, 2)
    top_k_idx = idx.reshape(4, 8192, 2)
    return dispatch_weights, top_k_idx, aux
